# revision 18
# baseline (speedup 1.0000x reference)
"""Adjacency-aware multi-head attention on 8 trn2 NeuronCores.

Math (per b, head k):
  Q = h[b] @ Wq[:, k] + bq[k]           [N, D]
  S[i, j] = (Q_i . K_j) / sqrt(D)
  P[j, i] = exp(S[i, j]) / sum_j exp(S[i, j])      (softmax over keys j)
  out[i, d] = sum_j P[j, i] * A[b, j, i] * V[j, d]

Sharding: 16 (b, head) pairs over 8 cores, 2 heads of the SAME b per core so
the A[b] stream is shared by both heads.

Device dataflow ([j, i] "transposed" layout so A needs no transpose):
  S^T[j-tile, i-chunk] on PE (K^T tile stationary bf16, Q^T moving bf16)
  exp on ACT: PSUM -> SBUF bf16
  EA = E * A on DVE (bf16 tensor_tensor, 2x mode)
  Phase 2, 4 concurrent col-group streams into one PSUM tile:
    cols  0-31: outT_h0 += V_h0[j]^T @ EA_h0     (M=32)
    cols 32-63: outT_h1 += V_h1[j]^T @ EA_h1     (M=32)
    col  64: denom_h0 += ones^T @ E_h0           (M=1)
    col  96: denom_h1 += ones^T @ E_h1           (M=1)
Device returns [128, N]: rows 0-31 outT_h0, 32-63 outT_h1, row 64/96 the
softmax denominators.  Host does out = (outT / denom)^T plus the gather.
"""

import math
import os

import numpy as np
import ml_dtypes

B, N, IN_DIM = 2, 2048, 256
HEADS, D = 8, 32
NCORES = 8
HPC = 2              # heads per core
NJ = N // 128        # 16 j-tiles
NCH = 4              # i-chunks
CH = N // NCH        # 512
CORES_PER_B = NCORES // B

LAST_RESULTS = None  # BassKernelResults of the most recent kernel() call


def _build_bass():
    import concourse.mybir as mybir
    import concourse.tile as tile
    from concourse import bacc

    f32 = mybir.dt.float32
    bf16 = mybir.dt.bfloat16
    AF = mybir.ActivationFunctionType

    nc = bacc.Bacc("TRN2", target_bir_lowering=False, debug=False,
                   num_devices=NCORES)

    hT = nc.dram_tensor("hT", [IN_DIM, N], bf16, kind="ExternalInput").ap()
    Ab = nc.dram_tensor("Ab", [N, N], bf16, kind="ExternalInput").ap()
    wq = nc.dram_tensor("wq", [IN_DIM, HPC * D], bf16, kind="ExternalInput").ap()
    wk = nc.dram_tensor("wk", [IN_DIM, HPC * D], bf16, kind="ExternalInput").ap()
    wv = nc.dram_tensor("wv", [IN_DIM, HPC * D], bf16, kind="ExternalInput").ap()
    bq = nc.dram_tensor("bq", [HPC * D, 1], f32, kind="ExternalInput").ap()
    bk = nc.dram_tensor("bk", [HPC * D, 1], f32, kind="ExternalInput").ap()
    bvb = nc.dram_tensor("bvb", [128, HPC * D], f32, kind="ExternalInput").ap()
    o = nc.dram_tensor("o", [128, N], f32, kind="ExternalOutput").ap()

    SC = 1.0 / math.sqrt(D)

    with (
        tile.TileContext(nc) as tc,
        tc.tile_pool(name="const", bufs=1) as cpool,
        tc.tile_pool(name="ps", bufs=2, space="PSUM") as pspool,
        tc.tile_pool(name="pod", bufs=2, space="PSUM") as podpool,
        tc.tile_pool(name="apool", bufs=2) as apool,
        tc.tile_pool(name="epool", bufs=2) as epool,
        tc.tile_pool(name="eapool", bufs=2) as eapool,
        tc.tile_pool(name="opool", bufs=3) as opool,
    ):
        # ---- constants / inputs into SBUF
        hT_sb = cpool.tile([128, 2, N], bf16, tag="hT")
        nc.sync.dma_start(hT_sb, hT.rearrange("(s p) n -> p s n", p=128))
        w_sb = {}
        for name, ap in (("q", wq), ("k", wk), ("v", wv)):
            t = cpool.tile([128, 2, HPC * D], bf16, tag=f"w{name}")
            nc.sync.dma_start(t, ap.rearrange("(s p) m -> p s m", p=128))
            w_sb[name] = t
        bq_sb = cpool.tile([HPC * D, 1], f32, tag="bq")
        nc.sync.dma_start(bq_sb, bq)
        bk_sb = cpool.tile([HPC * D, 1], f32, tag="bk")
        nc.sync.dma_start(bk_sb, bk)
        bvb_sb = cpool.tile([128, HPC * D], f32, tag="bvb")
        nc.sync.dma_start(bvb_sb, bvb)
        ones_sb = cpool.tile([128, 1], bf16, tag="ones")
        nc.vector.memset(ones_sb, 1.0)

        QT = cpool.tile([HPC * D, N], bf16, tag="qt")   # [64, 2048]
        KT = cpool.tile([HPC * D, N], bf16, tag="kt")
        Vt = cpool.tile([128, NJ, HPC * D], bf16, tag="vt")  # j-tile t at [:, t, :]

        # ---- projections: QT/KT = W^T @ h^T (+bias, Q scaled by 1/sqrt(D))
        for bias_sb, scale, dst, wname in (
            (bq_sb, SC, QT, "q"),
            (bk_sb, 1.0, KT, "k"),
        ):
            import concourse.bass as bass
            bias_bcast = bass.AP(
                tensor=bias_sb.tensor, offset=bias_sb.offset,
                ap=[bias_sb.ap[0], [0, CH]],
            )
            for quarter in range(4):  # 512 columns at a time
                ps = pspool.tile([HPC * D, CH], f32, tag="ps")
                for s in range(2):
                    nc.tensor.matmul(
                        ps,
                        lhsT=w_sb[wname][:, s, :],
                        rhs=hT_sb[:, s, quarter * CH:(quarter + 1) * CH],
                        start=(s == 0), stop=(s == 1),
                    )
                nc.vector.scalar_tensor_tensor(
                    dst[:, quarter * CH:(quarter + 1) * CH], ps, scale,
                    bias_bcast,
                    op0=mybir.AluOpType.mult, op1=mybir.AluOpType.add,
                )

        # ---- projection V[j, d] for both heads (+bias via broadcast tile);
        #      emitted lazily (after chunk 0's S matmuls) so S starts early
        def emit_vproj():
            for t in range(NJ):
                ps = pspool.tile([128, HPC * D], f32, tag="ps")
                for s in range(2):
                    nc.tensor.matmul(
                        ps,
                        lhsT=hT_sb[:, s, t * 128:(t + 1) * 128],
                        rhs=w_sb["v"][:, s, :],
                        start=(s == 0), stop=(s == 1),
                    )
                nc.vector.tensor_add(Vt[:, t, :], ps, bvb_sb)

        # ---- main loop (software-pipelined: phase2 lags one chunk so the
        #      in-order PE queue never stalls waiting for exp/A-mult)
        A3 = Ab.rearrange("(t p) i -> p t i", p=128)

        import concourse.bass as bass

        def emit_phase2_tile(od, e_t, ea_t, t):
            # e/ea layout: column block (t * HPC + hh) * CH holds head hh,
            # j-tile t.  4 concurrent col-group streams into one PSUM tile.
            first, last = (t == 0), (t == NJ - 1)
            sh0 = slice((t * HPC) * CH, (t * HPC + 1) * CH)
            sh1 = slice((t * HPC + 1) * CH, (t * HPC + 2) * CH)
            nc.tensor.matmul(
                od[0:D, :], lhsT=Vt[:, t, 0:D], rhs=ea_t[:, sh0],
                start=first, stop=last, tile_position=(0, 0),
            )
            nc.tensor.matmul(
                od[D:2 * D, :], lhsT=Vt[:, t, D:2 * D], rhs=ea_t[:, sh1],
                start=first, stop=last, tile_position=(0, 32),
            )
            nc.tensor.matmul(
                od[64:65, :], lhsT=ones_sb, rhs=e_t[:, sh0],
                start=first, stop=last, tile_position=(0, 64),
            )
            nc.tensor.matmul(
                od[96:97, :], lhsT=ones_sb, rhs=e_t[:, sh1],
                start=first, stop=last, tile_position=(0, 96),
            )

        def drain_od(od, ch):
            o_sb = opool.tile([128, CH], f32, tag="o")
            nc.vector.tensor_copy(o_sb, od)
            nc.sync.dma_start(o[:, ch * CH:(ch + 1) * CH], o_sb)

        def emit_phase2(e_t, ea_t, ch):
            od = podpool.tile([128, CH], f32, tag="od")
            for t in range(NJ):
                emit_phase2_tile(od, e_t, ea_t, t)
            drain_od(od, ch)

        pending = None
        for ch in range(NCH):
            a_t = apool.tile([128, NJ * CH], bf16, tag="a")
            nc.sync.dma_start(
                a_t.rearrange("p (t i) -> p t i", i=CH),
                A3[:, :, ch * CH:(ch + 1) * CH],
            )
            e_t = epool.tile([128, NJ * HPC * CH], bf16, tag="e")
            ea_t = eapool.tile([128, NJ * HPC * CH], bf16, tag="ea")
            # S matmuls stream block b = t*2+hh; psum tiles hold GRP blocks
            # so each exp covers GRP*CH elements; both heads' S matmuls are
            # adjacent (different PE row-groups, run concurrently)
            GRP = 3
            nblocks = NJ * HPC
            ps = None
            for b in range(nblocks):
                t, hh = b // HPC, b % HPC
                g = b % GRP
                if g == 0:
                    gsz = min(GRP, nblocks - b)
                    ps = pspool.tile([128, GRP * CH], f32, tag="ps")
                nc.tensor.matmul(
                    ps[:, g * CH:(g + 1) * CH],
                    lhsT=KT[hh * D:(hh + 1) * D, t * 128:(t + 1) * 128],
                    rhs=QT[hh * D:(hh + 1) * D, ch * CH:(ch + 1) * CH],
                    start=True, stop=True,
                )
                if g == gsz - 1:
                    b0 = b - g
                    nc.scalar.activation(
                        e_t[:, b0 * CH:(b + 1) * CH], ps[:, :gsz * CH], AF.Exp)
            if ch == 0:
                emit_vproj()
            if pending is not None:
                emit_phase2(*pending)
                pending = None
            last = (ch == NCH - 1)
            if last:
                od = podpool.tile([128, CH], f32, tag="od")
            for t in range(NJ):
                sl2 = slice(t * HPC * CH, (t + 1) * HPC * CH)
                a_sl = a_t[:, t * CH:(t + 1) * CH]
                a_bcast = bass.AP(
                    tensor=a_sl.tensor, offset=a_sl.offset,
                    ap=[a_sl.ap[0], [0, HPC], a_sl.ap[1]],
                )
                nc.vector.tensor_mul(
                    ea_t[:, sl2].rearrange("p (h i) -> p h i", h=HPC),
                    e_t[:, sl2].rearrange("p (h i) -> p h i", h=HPC),
                    a_bcast,
                )
                if last:
                    emit_phase2_tile(od, e_t, ea_t, t)
            if last:
                drain_od(od, ch)
            else:
                pending = (e_t, ea_t, ch)

    nc.finalize()
    return nc


def kernel(h, A, Wq, bq, Wk, bk, Wv, bv):
    global LAST_RESULTS
    from concourse.bass_utils import run_bass_kernel_spmd

    h = np.asarray(h, np.float32)
    A = np.asarray(A, np.float32)
    Wq = np.asarray(Wq, np.float32)
    Wk = np.asarray(Wk, np.float32)
    Wv = np.asarray(Wv, np.float32)
    bq = np.asarray(bq, np.float32)
    bk = np.asarray(bk, np.float32)
    bv = np.asarray(bv, np.float32)

    hT = np.ascontiguousarray(h.transpose(0, 2, 1)).astype(ml_dtypes.bfloat16)
    Ab = np.ascontiguousarray(A.astype(ml_dtypes.bfloat16))  # [B, N, N]
    sc = np.float32(1.0 / math.sqrt(D))

    in_maps = []
    for c in range(NCORES):
        b = c // CORES_PER_B
        h0 = HPC * (c % CORES_PER_B)
        sl = slice(h0 * D, (h0 + HPC) * D)
        in_maps.append({
            "hT": hT[b],
            "Ab": Ab[b],
            "wq": np.ascontiguousarray(Wq[:, sl]).astype(ml_dtypes.bfloat16),
            "wk": np.ascontiguousarray(Wk[:, sl]).astype(ml_dtypes.bfloat16),
            "wv": np.ascontiguousarray(Wv[:, sl]).astype(ml_dtypes.bfloat16),
            "bq": np.ascontiguousarray((bq[sl] * sc).reshape(-1, 1)),
            "bk": np.ascontiguousarray(bk[sl].reshape(-1, 1)),
            "bvb": np.ascontiguousarray(np.tile(bv[sl][None, :], (128, 1))),
        })

    nc = _build_bass()
    res = run_bass_kernel_spmd(
        nc, in_maps, core_ids=list(range(NCORES)),
        trace=os.environ.get("BASS_TRACE", "0") == "1",
    )
    LAST_RESULTS = res

    out = np.empty((B, HEADS, N, D), np.float32)
    for c in range(NCORES):
        b = c // CORES_PER_B
        h0 = HPC * (c % CORES_PER_B)
        oo = res.results[c]["o"]                  # [128, N] f32
        for hh in range(HPC):
            num = oo[hh * D:(hh + 1) * D, :]      # [32, N] unnormalized out^T
            den = oo[64 + 32 * hh, :]             # [N]
            out[b, h0 + hh] = (num / den[None, :]).T
    return out
